# revision 1
# baseline (speedup 1.0000x reference)
"""Multi-head self-attention Trainium2 kernel (8 NeuronCores, head-parallel).

Problem: L=4096, F_IN=1024, H=16, DH=64, F_OUT=1024, fp32.
Sharding: 2 heads per core (tensor parallel over heads). Each core computes
its 2 heads' attention and its partial output projection; the host sums the
8 partials (the all-reduce of the sharding hint, done at gather time).

Numerics: projections run in fp32r (TF32-like, full PE rate, fp32 PSUM
accumulate); q/k/v are then stored bf16, and the attention/output matmuls
run in bf16 (cheap LDWEIGHTS via fast-weight-load) with fp32 PSUM
accumulation. The softmax denominator is summed from the *rounded*
attention weights (ones-column trick), so bf16 rounding largely cancels in
the normalization; measured output error vs the fp32 reference is ~4e-3
relative to the output absmax.

Per-core pipeline:
  1. qT,kT,vT [128,4096] = W.T @ x.T from pre-transposed x (host side),
     processed in 1024-col quarters with double-buffered x tiles; v is
     turned into natural [j, d] layout via PE transpose-mode.
  2. Per i-chunk (512), per j-tile (128): scoresT for both heads as
     row-packed K=64 matmuls (concurrent sub-array execution) -> one ACT
     exp over the [128,1024] psum pair (ScalarE is the phase-2 roofline:
     ~1.11us per j-tile) -> attn@v with a ones-column on the stationary
     operand so PSUM row 64 accumulates the softmax denominators.
  3. Normalize + output projection are interleaved into the *next*
     i-chunk's j-loop to hide under the exp roof: evacuate vals (bf16) and
     denominators (fp32), reciprocal_approx_fast, gpsimd partition
     broadcast, normalize, out-proj (both heads accumulate into one PSUM
     bank), DMA out.

Bias handling: bq/bk folded into the ACT bias at qT/kT evacuation; bv is
exact as a host-side constant (softmax rows sum to 1 => out += sum_h
bv_h @ Wo_h); bo added on host. Phase-1/2 overlap relies on Tile's
range-granular dependency tracking (scores for j-tiles of quarter q start
as soon as that quarter's kT/vx are written).
"""

import numpy as np

L, F_IN, H, DH, F_OUT = 4096, 1024, 16, 64, 1024
NCORES = 8
HPC = H // NCORES  # heads per core = 2
D2 = HPC * DH      # 128, per-core packed head dim

_BUILT = None


def _build():
    import os

    import concourse.bass as bass  # noqa: F401
    import concourse.mybir as mybir
    import concourse.tile as tile
    from concourse import bacc
    from concourse.masks import make_identity

    F = mybir.dt.float32
    FR = mybir.dt.float32r
    BF = mybir.dt.bfloat16
    Act = mybir.ActivationFunctionType

    nc = bacc.Bacc("TRN2", target_bir_lowering=False, debug=False)

    xT_d = nc.declare_dram_parameter("xT", [F_IN, L], F, isOutput=False)
    wq_d = nc.declare_dram_parameter("wq", [F_IN, D2], F, isOutput=False)
    wk_d = nc.declare_dram_parameter("wk", [F_IN, D2], F, isOutput=False)
    wv_d = nc.declare_dram_parameter("wv", [F_IN, D2], F, isOutput=False)
    bq_d = nc.declare_dram_parameter("bq", [D2], F, isOutput=False)
    bk_d = nc.declare_dram_parameter("bk", [D2], F, isOutput=False)
    wo0_d = nc.declare_dram_parameter("wo0", [DH, F_OUT], F, isOutput=False)
    wo1_d = nc.declare_dram_parameter("wo1", [DH, F_OUT], F, isOutput=False)
    out_d = nc.declare_dram_parameter("out", [L, F_OUT], F, isOutput=True)

    dbg = bool(os.environ.get("K_DEBUG"))
    if dbg:
        dbg_q = nc.declare_dram_parameter("dbg_q", [128, L], F, isOutput=True)
        dbg_k = nc.declare_dram_parameter("dbg_k", [128, L], F, isOutput=True)
        dbg_v = nc.declare_dram_parameter("dbg_v", [128, 32 * 65], F, isOutput=True)

    KT = F_IN // 128   # 8 f-tiles
    NI = L // 512      # 8 i-chunks
    NJ = L // 128      # 32 j-tiles
    QL = 1024          # quarter width in L
    NQ = L // QL       # 4 quarters

    with tile.TileContext(nc) as tc:
        with tc.tile_pool(name="persist", bufs=1) as pp:
            qT = pp.tile([128, L], BF, tag="qT")             # [d2, i]
            kT = pp.tile([128, L], BF, tag="kT")             # [d2, j]
            vx0 = pp.tile([128, NJ, DH + 1], BF, tag="vx0")  # [j_in, jt, d|1]
            vx1 = pp.tile([128, NJ, DH + 1], BF, tag="vx1")
            bq = pp.tile([128, 1], F, tag="bq")
            bk = pp.tile([128, 1], F, tag="bk")
            ones32 = pp.tile([128, NJ], F, tag="ones32")
            warm = pp.tile([1, 1], F, tag="warm")

            # pre-warm the exp table set while DMAs run
            nc.vector.memset(warm[:], 0.0)
            nc.scalar.activation(warm[:], warm[:], Act.Exp, scale=1.0)

            nc.vector.memset(ones32[:], 1.0)
            nc.vector.tensor_copy(vx0[:, :, DH:DH + 1], ones32[:, :, None])
            nc.vector.tensor_copy(vx1[:, :, DH:DH + 1], ones32[:, :, None])

            # Pools for the attention phase are opened before phase 1 is
            # emitted so the scheduler can overlap the phase-1 tail with
            # early score matmuls (PSUM: ps2s 4 + ps2v 2 + phase1 2 = 8).
            with tc.tile_pool(name="p2", bufs=1) as p2, \
                 tc.tile_pool(name="p2v", bufs=2) as p2v, \
                 tc.tile_pool(name="expp", bufs=6) as pe, \
                 tc.tile_pool(name="outp", bufs=4) as po, \
                 tc.tile_pool(name="ps2s", bufs=2, space="PSUM") as ps2s, \
                 tc.tile_pool(name="ps2v", bufs=1, space="PSUM") as ps2v:
                wo0 = p2.tile([DH, F_OUT], FR, tag="wo0")
                wo1 = p2.tile([DH, F_OUT], FR, tag="wo1")
                nc.sync.dma_start(out=wo0[:], in_=wo0_d.ap().bitcast(FR))
                nc.sync.dma_start(out=wo1[:], in_=wo1_d.ap().bitcast(FR))

                # ---- Phase 1: QKV projections over 4 quarters of L ----
                with tc.tile_pool(name="p1w", bufs=1) as p1w, \
                     tc.tile_pool(name="p1x", bufs=2) as p1x, \
                     tc.tile_pool(name="ps1", bufs=2, space="PSUM") as ps1:
                    wq = p1w.tile([128, KT, D2], FR, tag="wq")
                    wk = p1w.tile([128, KT, D2], FR, tag="wk")
                    wv = p1w.tile([128, KT, D2], FR, tag="wv")
                    ident = p1w.tile([128, 128], F, tag="ident")
                    for wt, wd in ((wk, wk_d), (wv, wv_d), (wq, wq_d)):
                        nc.sync.dma_start(
                            out=wt[:],
                            in_=wd.ap().rearrange(
                                "(k p) d -> p k d", p=128).bitcast(FR),
                        )
                    make_identity(nc, ident[:])
                    nc.sync.dma_start(out=bq[:], in_=bq_d.ap()[:, None])
                    nc.sync.dma_start(out=bk[:], in_=bk_d.ap()[:, None])

                    def proj(wt, dst, bias, xt, c0, g0):
                        ps = ps1.tile([128, 512], F, tag="ps1")
                        for kt in range(KT):
                            nc.tensor.matmul(
                                ps[:], wt[:, kt, :], xt[:, kt, c0:c0 + 512],
                                start=(kt == 0), stop=(kt == KT - 1),
                            )
                        if bias is not None:
                            nc.scalar.activation(
                                dst[:, g0:g0 + 512], ps[:], Act.Identity,
                                bias=bias[:], scale=1.0,
                            )
                        else:
                            nc.scalar.copy(dst[:, c0:c0 + 512], ps[:])

                    for qq in range(NQ):
                        l0 = qq * QL
                        xt = p1x.tile([128, KT, QL], FR, tag="xt")
                        for kt in range(KT):
                            for hf in range(2):
                                h0 = hf * (QL // 2)
                                nc.sync.dma_start(
                                    out=xt[:, kt, h0:h0 + QL // 2],
                                    in_=xT_d.ap()[kt * 128:(kt + 1) * 128,
                                                  l0 + h0:l0 + h0 + QL // 2
                                                  ].bitcast(FR),
                                )
                        vTq = p1x.tile([128, QL], F, tag="vTq")
                        # k and v unlock this quarter's score matmuls; qT is
                        # ramp-critical only in quarter 0 (i-chunk 0 columns),
                        # so later quarters emit the q chunks last.
                        for ch in range(QL // 512):
                            proj(wk, kT, bk, xt, ch * 512, l0 + ch * 512)
                            proj(wv, vTq, None, xt, ch * 512, ch * 512)
                            if qq == 0:
                                proj(wq, qT, bq, xt, ch * 512, l0 + ch * 512)
                        for jl in range(QL // 128):
                            jt = qq * (QL // 128) + jl
                            pt = ps1.tile([128, 512], F, tag="ps1")
                            nc.tensor.transpose(
                                pt[:, 0:128],
                                vTq[:, jl * 128:(jl + 1) * 128], ident[:])
                            nc.vector.tensor_copy(vx0[:, jt, 0:DH], pt[:, 0:DH])
                            nc.vector.tensor_copy(vx1[:, jt, 0:DH],
                                                  pt[:, DH:D2])
                        if qq != 0:
                            for ch in range(QL // 512):
                                proj(wq, qT, bq, xt, ch * 512, l0 + ch * 512)

                if dbg:
                    nc.sync.dma_start(out=dbg_q.ap(), in_=qT[:].bitcast(F))
                    nc.sync.dma_start(out=dbg_k.ap(), in_=kT[:].bitcast(F))
                    nc.sync.dma_start(
                        out=dbg_v.ap(),
                        in_=vx0[:].bitcast(F).rearrange("p a b -> p (a b)"))

                # ---- Phase 2+3: attention, interleaved normalize/out-proj ----
                with tc.tile_pool(name="ps2o", bufs=2, space="PSUM") as ps2o:
                    _phase2(nc, ps2s, ps2v, ps2o, pe, po, p2v,
                            qT, kT, vx0, vx1, wo0, wo1, out_d,
                            NI, NJ, F, FR, BF, Act)

    nc.compile()
    return nc


def _phase2(nc, ps2s, ps2v, ps2o, pe, po, p2v, qT, kT, vx0, vx1, wo0, wo1,
            out_d, NI, NJ, F, FR, BF, Act):
    def norm_unit(ic, p0, p1):
        # reciprocal + broadcast of the softmax denominators for chunk ic
        for (va, _), tg in ((p0, "0"), (p1, "1")):
            sh = p2v.tile([1, 512], F, tag="sh" + tg)
            rc = p2v.tile([1, 512], F, tag="rc" + tg)
            rb = p2v.tile([DH, 512], F, tag="rb" + tg)
            nc.sync.dma_start(out=sh[:], in_=va[DH:DH + 1, :].bitcast(F))
            nc.vector.reciprocal_approx_fast(out=rc[:], in_=sh[:])
            nc.gpsimd.partition_broadcast(rb[:], rc[:], channels=DH)
            nc.vector.tensor_mul(va[0:DH, :], va[0:DH, :], rb[:])

    def oproj_unit(ic, p0, p1, iw, fc, evac="v"):
        # one output-projection tile of chunk ic
        isl = slice(iw * 128, (iw + 1) * 128)
        r0 = ic * 512 + iw * 128
        f0 = fc * 512
        pso = ps2o.tile([128, 512], F, tag="pso")
        nc.tensor.matmul(
            pso[:], p0[0][0:DH, isl], wo0[:, f0:f0 + 512],
            start=True, stop=False,
        )
        nc.tensor.matmul(
            pso[:], p1[0][0:DH, isl], wo1[:, f0:f0 + 512],
            start=False, stop=True,
        )
        ot = po.tile([128, 512], F, tag="ot")
        if evac == "s":
            nc.scalar.copy(ot[:], pso[:])
        else:
            nc.vector.tensor_copy(ot[:], pso[:])
        nc.sync.dma_start(
            out=out_d.ap()[r0:r0 + 128, f0:f0 + 512], in_=ot[:])

    def emit_scores(ic, jt):
        i0 = ic * 512
        j0 = jt * 128
        ps = ps2s.tile([128, 1024], F, tag="pss")
        nc.tensor.matmul(
            ps[:, 0:512], kT[0:64, j0:j0 + 128], qT[0:64, i0:i0 + 512],
            start=True, stop=True, tile_position=(0, 0),
        )
        nc.tensor.matmul(
            ps[:, 512:1024], kT[64:128, j0:j0 + 128],
            qT[64:128, i0:i0 + 512],
            start=True, stop=True, tile_position=(64, 0),
        )
        return ps

    # One-step score skew: scores for step n+1 are emitted between exp(n)
    # and vals(n), so on TensorE's FIFO they are not blocked behind the
    # exp(n)-gated vals, and exp(n+1)'s input is ready a full step early.
    pending = None  # (ic, va0, va1) with normalize+out-proj still to emit
    ps_next = emit_scores(0, 0)
    for ic in range(NI):
        pv0 = ps2v.tile([DH + 1, 512], F, tag="pv0")
        pv1 = ps2v.tile([DH + 1, 512], F, tag="pv1")
        # out-proj units of the previous chunk, spread through this jt loop
        units = []
        if pending is not None:
            pic, pp0, pp1 = pending
            units = [(pic, pp0, pp1, iw, fc)
                     for iw in range(4) for fc in range(F_OUT // 512)]
        for jt in range(NJ):
            ps = ps_next
            eT = pe.tile([128, 1024], BF, tag="eT")
            nc.scalar.activation(eT[:], ps[:], Act.Exp, scale=0.125)
            n = ic * NJ + jt
            if n + 1 < NI * NJ:
                ps_next = emit_scores((n + 1) // NJ, (n + 1) % NJ)
            nc.tensor.matmul(
                pv0[:], vx0[:, jt, :], eT[:, 0:512],
                start=(jt == 0), stop=(jt == NJ - 1),
            )
            nc.tensor.matmul(
                pv1[:], vx1[:, jt, :], eT[:, 512:1024],
                start=(jt == 0), stop=(jt == NJ - 1),
            )
            if jt == 0 and pending is not None:
                norm_unit(*pending)
            if jt % 2 == 1 and jt < 17 and units:
                oproj_unit(*units.pop(0))
        for u in units:
            oproj_unit(*u)

        # evacuate this chunk's vals+denominators in one fp32r copy per
        # head: the next chunk's first vals matmul reclaims the PSUM bank
        # after ~0.7us instead of waiting a 4-copy chain
        va0 = p2v.tile([DH + 1, 512], FR, tag="va0")
        va1 = p2v.tile([DH + 1, 512], FR, tag="va1")
        nc.vector.tensor_copy(va0[:], pv0[:])
        nc.vector.tensor_copy(va1[:], pv1[:])
        pending = (ic, (va0, None), (va1, None))

    norm_unit(*pending)
    pic, pp0, pp1 = pending
    for iw in range(4):
        for fc in range(F_OUT // 512):
            oproj_unit(pic, pp0, pp1, iw, fc, evac="s" if fc == 0 else "v")


def _get_built():
    global _BUILT
    if _BUILT is None:
        _BUILT = _build()
    return _BUILT


def kernel(x, Wq, bq, Wk, bk, Wv, bv, Wo, bo):
    from concourse.bass_utils import run_bass_kernel_spmd

    x = np.ascontiguousarray(np.asarray(x, dtype=np.float32))
    Wq = np.asarray(Wq, dtype=np.float32)
    Wk = np.asarray(Wk, dtype=np.float32)
    Wv = np.asarray(Wv, dtype=np.float32)
    Wo = np.asarray(Wo, dtype=np.float32)
    bq = np.asarray(bq, dtype=np.float32)
    bk = np.asarray(bk, dtype=np.float32)
    bv = np.asarray(bv, dtype=np.float32)
    bo = np.asarray(bo, dtype=np.float32)

    nc = _get_built()

    xT = np.ascontiguousarray(x.T)  # [F_IN, L]
    in_maps = []
    for c in range(NCORES):
        hs = slice(c * HPC, (c + 1) * HPC)
        in_maps.append({
            "xT": xT,
            "wq": np.ascontiguousarray(Wq[:, hs, :].reshape(F_IN, D2)),
            "wk": np.ascontiguousarray(Wk[:, hs, :].reshape(F_IN, D2)),
            "wv": np.ascontiguousarray(Wv[:, hs, :].reshape(F_IN, D2)),
            "bq": np.ascontiguousarray(bq[hs].reshape(D2)),
            "bk": np.ascontiguousarray(bk[hs].reshape(D2)),
            "wo0": np.ascontiguousarray(Wo[c * HPC]),
            "wo1": np.ascontiguousarray(Wo[c * HPC + 1]),
        })

    res = run_bass_kernel_spmd(nc, in_maps, list(range(NCORES)))
    acc = np.zeros((L, F_OUT), dtype=np.float64)
    for c in range(NCORES):
        acc += res.results[c]["out"].astype(np.float64)
    # bv contribution (softmax rows sum to 1) + bo, both exact on host
    acc += (bv.reshape(1, H * DH).astype(np.float64)
            @ Wo.reshape(H * DH, F_OUT).astype(np.float64))
    acc += bo.astype(np.float64)
    return acc.astype(np.float32)

